# revision 20
# baseline (speedup 1.0000x reference)
"""GATv2 (2-layer) + global-mean-pool + MLP head on 8 Trainium2 NeuronCores.

Self-contained: host preprocessing (numpy) + Bass/Tile program + SPMD run.

Strategy v2 (dst-sharded graph parallel, sharded table build + AllGather):
  - Nodes/edges sharded across 8 cores by destination-node windows of 128.
  - Layer math folded so each edge needs ONE gathered fp16 table row:
      table row n = [att*xl[n] (feature-permuted, +att first) | att.xl[n] | 1 | pad]
    using leaky_relu(z,0.2) = 0.6 z + 0.4 |z|:
      e = 0.6(pl_s + pr_d) + 0.4(sum_{att>0}|u| - sum_{att<=0}|u|),  u = att*m
    Aggregation runs on att-scaled rows, un-scaled by 1/att afterwards.
  - Each core builds ONLY its local table shard [128, NWC*TW] in SBUF
    (partition-major), writes it with one big-descriptor DMA, and an fp16
    AllGather replicates the full table to every core.  Gathers use
    host-precomputed FLAT element offsets into the gathered layout.
  - ONE merged indirect DMA per window gathers all 7 chunks (896 rows).
  - xr[dst] per edge expanded on-chip: one-hot built from dst values +
    iota (is_equal); its transpose comes from PE transposes (fp16 PSUM).
  - Pad edges get dst_rel=128: their one-hot column is all-zero, so they
    are masked from softmax numerator+denominator for free (no ebp bias).
  - exp() without segment-max (validated: e in [-2.6, 3.6]); denominators
    clamped before reciprocal.  Edge-phase leaky-ReLUs run on the vector
    engine as max(z, 0.01 z) so the scalar engine only ever runs Exp
    (no activation-table thrash).
  - Layer outputs are kept transposed in SBUF (h2T) and feed layer-2's
    local shard build directly; the only collectives are the two table
    AllGathers and one tiny AllReduce of the [128f x 128g] pool sums.
"""
import numpy as np

P = 128          # partitions / window size / chunk size
TW = 132         # table row width: 128 feats + pl + 1.0 + 2 pad
TWG = 136        # gathered-chunk stride in SBUF: TW + 4 gap elements.
                 # The gap keeps the gather's dest AP non-collapsible so the
                 # DGE emits one descriptor (= consumes one index) per chunk
                 # row instead of fusing each partition row into one segment.

FULL_CFG = dict(N=100000, DIN=64, H=128, E0=600000, G=128, DOUT=16, NCORES=8)

DEBUG = False    # adds intermediate-dump outputs for bring-up


# ---------------------------------------------------------------------------
# host preprocessing
# ---------------------------------------------------------------------------

def _sign_perm(att):
    pos = np.where(att > 0)[0]
    neg = np.where(att <= 0)[0]
    return np.concatenate([pos, neg]), len(pos)


def _aug_weights(W, b, att, perm, attp):
    H = W.shape[0]
    Wa = np.zeros((H, TW), np.float32)
    ba = np.zeros((TW,), np.float32)
    Wa[:, :H] = W[:, perm] * attp[None, :]
    ba[:H] = b[perm] * attp
    Wa[:, H] = W @ att
    ba[H] = att @ b
    ba[H + 1] = 1.0
    return Wa, ba


def host_prep(inputs, cfg):
    N, DIN, H, E0, G, NCORES = (cfg["N"], cfg["DIN"], cfg["H"], cfg["E0"],
                                cfg["G"], cfg["NCORES"])
    x = np.asarray(inputs["x"], np.float32)
    ei = np.asarray(inputs["edge_index"]).astype(np.int64)
    batch = np.asarray(inputs["batch"]).astype(np.int64)
    get = lambda k: np.asarray(inputs[k], np.float32)
    f16 = np.float16

    NWT = (N + P - 1) // P
    NWC = (NWT + NCORES - 1) // NCORES
    NLOC = NWC * P

    att1, att2 = get("att1"), get("att2")
    perm1, Pp1 = _sign_perm(att1)
    perm2, Pp2 = _sign_perm(att2)
    attp1 = att1[perm1].astype(np.float32)
    attp2 = att2[perm2].astype(np.float32)

    Wl1a, bl1a = _aug_weights(get("Wl1"), get("bl1"), att1, perm1, attp1)
    Wr1a, br1a = _aug_weights(get("Wr1"), get("br1"), att1, perm1, attp1)
    br1a[H + 1] = 0.0
    Wl2a, bl2a = _aug_weights(get("Wl2")[perm1, :], get("bl2"), att2, perm2, attp2)
    Wr2a, br2a = _aug_weights(get("Wr2")[perm1, :], get("br2"), att2, perm2, attp2)
    br2a[H + 1] = 0.0

    weights = {
        "wnfc": get("W_nfc").astype(f16),
        "bnfc": get("b_nfc").reshape(H, 1),
        "wl1a": Wl1a.astype(f16), "wr1a": Wr1a.astype(f16),
        "bl1B": np.tile(bl1a, (P, 1)), "br1B": np.tile(br1a, (P, 1)),
        "wl2a": Wl2a.astype(f16), "wr2a": Wr2a.astype(f16),
        "bl2B": np.tile(bl2a, (P, 1)), "br2B": np.tile(br2a, (P, 1)),
        "arec1B": np.tile((1.0 / attp1), (P, 1)),
        "arec2B": np.tile((1.0 / attp2), (P, 1)),
        "bias1B": np.tile(get("bias1")[perm1], (P, 1)),
        "bias2B": np.tile(get("bias2")[perm2], (P, 1)),
        "wfc1": get("W_fc1")[perm2, :].astype(f16),
        "bfc1": get("b_fc1").reshape(1, -1).astype(f16),
        "wfc2": get("W_fc2").astype(f16),
        "bfc2": get("b_fc2").reshape(1, -1).astype(f16),
    }
    assert np.abs(attp1).min() > 1e-12 and np.abs(attp2).min() > 1e-12

    # local x, transposed, per core
    xT = np.zeros((DIN, NCORES * NLOC), f16)
    xT[:, :N] = x.T.astype(f16)
    xTloc = [np.ascontiguousarray(xT[:, c * NLOC:(c + 1) * NLOC])
             for c in range(NCORES)]

    # row index of node n in the AllGathered table layout
    # [NCORES*P*NWC, TW]:  (rank, p, w) -> (rank*P + p)*NWC + w
    def node_flat(n):
        r = n // NLOC
        rem = n % NLOC
        w = rem // P
        p = rem % P
        return ((r * P + p) * NWC + w).astype(np.int32)

    # --- edges (self-loops are NOT appended; they become chunk 0/window) ---
    src0, dst0 = ei[0], ei[1]
    per_core = []
    for c in range(NCORES):
        lo, hi = c * NLOC, min((c + 1) * NLOC, N)
        sel = (dst0 >= lo) & (dst0 < hi)
        s, d = src0[sel], dst0[sel]
        o = np.argsort(d, kind="stable")
        per_core.append((s[o], d[o] - lo))

    cnt = np.zeros((NCORES, NWC), np.int64)
    for c in range(NCORES):
        _, dl = per_core[c]
        cnt[c] = np.bincount(dl // P, minlength=NWC)
    # chunk 0 = self chunk; then real-edge chunks
    cw = 1 + np.ceil(cnt.max(axis=0) / P).astype(np.int64)      # [NWC]
    k0 = np.concatenate([[0], np.cumsum(cw)])
    K = int(k0[-1])
    CWMAX = int(cw.max())
    assert CWMAX <= 9, f"CWMAX={CWMAX} exceeds 3-PSUM-bank layout"

    src_idx = np.zeros((NCORES, P, K), np.int32)    # FLAT element offsets
    dst_rel = np.full((NCORES, P, K), 128.0, f16)   # 128 = masked pad
    for c in range(NCORES):
        lo, hi = c * NLOC, min((c + 1) * NLOC, N)
        s, dl = per_core[c]
        w = dl // P
        starts = np.searchsorted(w, np.arange(NWC), side="left")
        ends = np.searchsorted(w, np.arange(NWC), side="right")
        for wi in range(NWC):
            # self chunk: slot p -> node p of window (real nodes only)
            nreal = max(0, min(hi - (lo + wi * P), P))
            pr = np.arange(nreal)
            dst_rel[c, pr, k0[wi]] = pr.astype(f16)
            # real edges from chunk k0[wi]+1
            a, b = int(starts[wi]), int(ends[wi])
            n = b - a
            j = np.arange(n)
            ch = k0[wi] + 1 + j // P
            pr = j % P
            src_idx[c, pr, ch] = node_flat(s[a:b])
            dst_rel[c, pr, ch] = (dl[a:b] % P).astype(f16)
            # pad slots keep src_idx 0 (row 0) and dst_rel 128 (masked)

    gmask = np.zeros((NCORES, P, NWC * G), f16)
    for c in range(NCORES):
        lo, hi = c * NLOC, min((c + 1) * NLOC, N)
        for wi in range(NWC):
            nlo = lo + wi * P
            nn = max(0, min(hi - nlo, P))
            if nn <= 0:
                continue
            gmask[c, np.arange(nn), wi * G + batch[nlo:nlo + nn]] = 1.0
    counts = np.bincount(batch, minlength=G).astype(np.float32)
    countsRecipB = np.tile(1.0 / np.maximum(counts, 1.0), (P, 1)).astype(np.float32)

    meta = dict(cfg=cfg, NWC=NWC, NLOC=NLOC, K=K, CWMAX=CWMAX,
                cw=cw.tolist(), k0=k0.tolist(), Pp1=Pp1, Pp2=Pp2)
    data = dict(weights=weights, xTloc=xTloc, src_idx=src_idx,
                dst_rel=dst_rel, gmask=gmask, countsRecipB=countsRecipB)
    return meta, data


# ---------------------------------------------------------------------------
# device program
# ---------------------------------------------------------------------------

def build_program(meta):
    import concourse.bass as bass
    import concourse.bacc as bacc
    import concourse.tile as tile
    import concourse.mybir as mybir
    from concourse.masks import make_identity

    cfg = meta["cfg"]
    N, DIN, H, G, DOUT, NCORES = (cfg["N"], cfg["DIN"], cfg["H"], cfg["G"],
                                  cfg["DOUT"], cfg["NCORES"])
    NWC, NLOC, K, CWMAX = meta["NWC"], meta["NLOC"], meta["K"], meta["CWMAX"]
    cw, k0 = meta["cw"], meta["k0"]
    f32 = mybir.dt.float32
    f16 = mybir.dt.float16
    AF = mybir.ActivationFunctionType
    OP = mybir.AluOpType

    nc = bacc.Bacc("TRN2", target_bir_lowering=False, debug=False,
                   num_devices=NCORES)

    d_xTloc = nc.dram_tensor("xTloc", [DIN, NLOC], f16, kind="ExternalInput")
    d_src = nc.dram_tensor("src_idx", [P, K], mybir.dt.int32, kind="ExternalInput")
    d_dst = nc.dram_tensor("dst_rel", [P, K], f16, kind="ExternalInput")
    d_gmask = nc.dram_tensor("gmask", [P, NWC * G], f16, kind="ExternalInput")
    d_crecip = nc.dram_tensor("countsRecipB", [P, G], f32, kind="ExternalInput")
    wnames = {
        "wnfc": ([DIN, H], f16), "bnfc": ([H, 1], f32),
        "wl1a": ([H, TW], f16), "wr1a": ([H, TW], f16),
        "bl1B": ([P, TW], f32), "br1B": ([P, TW], f32),
        "wl2a": ([H, TW], f16), "wr2a": ([H, TW], f16),
        "bl2B": ([P, TW], f32), "br2B": ([P, TW], f32),
        "arec1B": ([P, H], f32), "arec2B": ([P, H], f32),
        "bias1B": ([P, H], f32), "bias2B": ([P, H], f32),
        "wfc1": ([H, 32], f16), "bfc1": ([1, 32], f16),
        "wfc2": ([32, DOUT], f16), "bfc2": ([1, DOUT], f16),
    }
    d_w = {k: nc.dram_tensor(k, shp, dt, kind="ExternalInput")
           for k, (shp, dt) in wnames.items()}
    d_out = nc.dram_tensor("out", [G, DOUT], f32, kind="ExternalOutput")
    if DEBUG:
        d_dbg_gt = nc.dram_tensor("dbg_gt", [P, (CWMAX - 1) * TW], f16,
                                  kind="ExternalOutput")
        d_dbg_tab = nc.dram_tensor("dbg_tab", [P, TW], f16,
                                   kind="ExternalOutput")
        d_dbg_h2T = nc.dram_tensor("dbg_h2T", [P, NLOC], f16,
                                   kind="ExternalOutput")

    NROWS = NCORES * P * NWC
    d_tabloc = nc.dram_tensor("tabloc", [P, NWC * TW], f16, kind="Internal")
    d_tab1full = nc.dram_tensor("tab1full", [NROWS, TW], f16, kind="Internal",
                                addr_space="Shared")
    d_tab2full = nc.dram_tensor("tab2full", [NROWS, TW], f16, kind="Internal",
                                addr_space="Shared")
    d_gsin = nc.dram_tensor("gsin", [P, G], f32, kind="Internal")
    d_gsout = nc.dram_tensor("gsout", [P, G], f32, kind="Internal",
                             addr_space="Shared")

    def bcast_last(ap2d, c, j):
        return bass.AP(ap2d.tensor, ap2d.offset,
                       [list(ap2d.ap[0]), list(ap2d.ap[1]), [0, j]])

    def bcast_mid(ap2d, c):
        return bass.AP(ap2d.tensor, ap2d.offset,
                       [list(ap2d.ap[0]), [0, c], list(ap2d.ap[1])])

    # [p, bank, slot-in-bank, f] view of the 3-bank m' PSUM tile
    def psm_view(psm_ap, nb, ns, fstart, flen):
        return bass.AP(psm_ap.tensor, psm_ap.offset + fstart,
                       [list(psm_ap.ap[0]), [512, nb], [TW, ns], [1, flen]])

    with tile.TileContext(nc) as tc:
        with tc.tile_pool(name="const", bufs=1) as cpool:
            identf = cpool.tile([P, P], f32)
            make_identity(nc, identf[:, :])
            ident = cpool.tile([P, P], f16)
            nc.vector.tensor_copy(ident[:, :], identf[:, :])
            iotaI = cpool.tile([P, P], mybir.dt.int32)
            nc.gpsimd.iota(iotaI[:, :], pattern=[[1, P]], base=0,
                           channel_multiplier=0)
            iotaF = cpool.tile([P, P], f16)
            nc.vector.tensor_copy(iotaF[:, :], iotaI[:, :])
            ones1 = cpool.tile([1, P], f16)
            nc.vector.memset(ones1[:, :], 1.0)

            w_sb = {}
            for k, (shp, dt) in wnames.items():
                w_sb[k] = cpool.tile(shp, dt, name=f"w_{k}", tag=f"w_{k}")
                nc.sync.dma_start(out=w_sb[k][:, :], in_=d_w[k][:, :])
            src_sb = cpool.tile([P, K], mybir.dt.int32)
            nc.sync.dma_start(out=src_sb[:, :], in_=d_src[:, :])
            dst_sb = cpool.tile([P, K], f16)
            nc.sync.dma_start(out=dst_sb[:, :], in_=d_dst[:, :])
            gmask_sb = cpool.tile([P, NWC * G], f16)
            nc.sync.dma_start(out=gmask_sb[:, :], in_=d_gmask[:, :])
            crecip_sb = cpool.tile([P, G], f32)
            nc.sync.dma_start(out=crecip_sb[:, :], in_=d_crecip[:, :])

            with tc.tile_pool(name="big", bufs=1) as bigp:
                tabloc_sb = bigp.tile([P, NWC * TW], f16, tag="tabloc")
                xr_sb = bigp.tile([P, NWC * TW], f16, tag="xr")
                h2T_sb = bigp.tile([P, NLOC], f16, tag="h2T")

                # ============ phase A: local table shard + xr products =====
                def phase_A(layer, lhsT_fn, d_tabfull):
                    wla = w_sb["wl1a" if layer == 1 else "wl2a"]
                    blB = w_sb["bl1B" if layer == 1 else "bl2B"]
                    wra = w_sb["wr1a" if layer == 1 else "wr2a"]
                    brB = w_sb["br1B" if layer == 1 else "br2B"]
                    with tc.tile_pool(name=f"pA{layer}", bufs=2,
                                      space="PSUM") as pp:
                        for w in range(NWC):
                            ps = pp.tile([P, TW], f32, tag="tab")
                            nc.tensor.matmul(out=ps[:, :], lhsT=lhsT_fn(w),
                                             rhs=wla[:, :], start=True,
                                             stop=True)
                            nc.vector.scalar_tensor_tensor(
                                out=tabloc_sb[:, w * TW:(w + 1) * TW],
                                in0=ps[:, :], scalar=1.0, in1=blB[:, :],
                                op0=OP.mult, op1=OP.add)
                        # one big-descriptor write + fp16 AllGather
                        nc.sync.dma_start(out=d_tabloc[:, :],
                                          in_=tabloc_sb[:, :])
                        tf = d_tabfull[:, :]
                        tf2 = bass.AP(tf.tensor, 0,
                                      [[P * NWC * TW, NCORES],
                                       [1, P * NWC * TW]])
                        nc.gpsimd.collective_compute(
                            "AllGather", OP.bypass,
                            replica_groups=[list(range(NCORES))],
                            ins=[d_tabloc[:, :]], outs=[tf2])
                        # xr products overlap with the AllGather
                        for w in range(NWC):
                            ps = pp.tile([P, TW], f32, tag="xr")
                            nc.tensor.matmul(out=ps[:, :], lhsT=lhsT_fn(w),
                                             rhs=wra[:, :], start=True,
                                             stop=True)
                            nc.vector.scalar_tensor_tensor(
                                out=xr_sb[:, w * TW:(w + 1) * TW],
                                in0=ps[:, :], scalar=1.0, in1=brB[:, :],
                                op0=OP.mult, op1=OP.add)

                # ---- layer 1 inputs: hx = lrelu(x @ Wnfc + b), local only
                with (
                    tc.tile_pool(name="xf", bufs=1) as xfp,
                    tc.tile_pool(name="hx", bufs=1) as hxp,
                    tc.tile_pool(name="nfps", bufs=2, space="PSUM") as npp,
                ):
                    xfull = xfp.tile([DIN, NLOC], f16)
                    nc.sync.dma_start(out=xfull[:, :], in_=d_xTloc[:, :])
                    hxloc = hxp.tile([P, NLOC], f16)
                    NG = (NLOC + 511) // 512
                    for g in range(NG):
                        g0 = g * 512
                        gl = min(512, NLOC - g0)
                        psn = npp.tile([P, 512], f32, tag="nfc")
                        nc.tensor.matmul(out=psn[:, :gl],
                                         lhsT=w_sb["wnfc"][:, :],
                                         rhs=xfull[:, g0:g0 + gl],
                                         start=True, stop=True)
                        nc.scalar.activation(out=hxloc[:, g0:g0 + gl],
                                             in_=psn[:, :gl], func=AF.Lrelu,
                                             bias=w_sb["bnfc"][:, :],
                                             scale=1.0, alpha=0.01)

                    phase_A(1, lambda w: hxloc[:, w * P:(w + 1) * P],
                            d_tab1full)

                # ============ edge phase ============
                def edge_phase(layer, d_tabfull, Pp, h_out_cb, extra_psum):
                    arecB = w_sb["arec1B" if layer == 1 else "arec2B"]
                    biasB = w_sb["bias1B" if layer == 1 else "bias2B"]
                    with (
                        tc.tile_pool(name=f"eg{layer}", bufs=3) as gp,
                        tc.tile_pool(name=f"eo{layer}", bufs=3) as ohp,
                        tc.tile_pool(name=f"es{layer}", bufs=4) as ssp,
                        tc.tile_pool(name=f"epm{layer}", bufs=1,
                                     space="PSUM") as ppm,
                        tc.tile_pool(name=f"epoh{layer}", bufs=1,
                                     space="PSUM") as ppoh,
                        tc.tile_pool(name=f"epo{layer}", bufs=2,
                                     space="PSUM") as ppo,
                    ):
                        for w in range(NWC):
                            c = cw[w]
                            g = c - 1
                            ks = k0[w]
                            tw0 = w * TW
                            xrw = xr_sb[:, tw0:tw0 + TW]
                            tabw = tabloc_sb[:, tw0:tw0 + TW]
                            # ---- merged gather of all non-self chunks
                            gt = gp.tile([P, max(g, 1) * TW], f16, tag="g")
                            for j in range(g):
                                nc.gpsimd.indirect_dma_start(
                                    out=gt[:, j * TW:(j + 1) * TW],
                                    out_offset=None,
                                    in_=d_tabfull[:, :],
                                    in_offset=bass.IndirectOffsetOnAxis(
                                        ap=src_sb[:, ks + 1 + j:ks + 2 + j],
                                        axis=0))
                            if DEBUG and layer == 1 and w == 0:
                                nc.sync.dma_start(out=d_dbg_gt[:, :g * TW],
                                                  in_=gt[:, :g * TW])
                                tabrows = ssp.tile([P, TW], f16, tag="dbgt")
                                nc.sync.dma_start(
                                    out=tabrows[:, :],
                                    in_=bass.AP(d_tabfull[:, :].tensor,
                                                (3 * P * NWC + 5) * TW,
                                                [[NWC * TW, P], [1, TW]]))
                                nc.sync.dma_start(out=d_dbg_tab[:, :],
                                                  in_=tabrows[:, :])
                            # ---- one-hot [P, c, 128] over ALL chunks
                            oh = ohp.tile([P, CWMAX * P], f16, tag="oh")
                            oh3 = oh[:, :c * P].rearrange("p (c j) -> p c j",
                                                          j=P)
                            nc.vector.tensor_tensor(
                                out=oh3,
                                in0=bcast_last(dst_sb[:, ks:ks + c], c, P),
                                in1=bcast_mid(iotaF[:, :], c),
                                op=OP.is_equal)
                            # ---- transposed one-hots via PE (fp16 PSUM)
                            ohT = ohp.tile([P, max(g, 1) * P], f16, tag="ohT")
                            if g > 0:
                                psoh = ppoh.tile([P, 1024], f16, tag="psoh")
                                for j in range(g):
                                    nc.tensor.transpose(
                                        out=psoh[:, j * P:(j + 1) * P],
                                        in_=oh[:, (j + 1) * P:(j + 2) * P],
                                        identity=ident[:, :])
                                nc.vector.tensor_copy(ohT[:, :g * P],
                                                      psoh[:, :g * P])
                            # ---- m' into 3 PSUM banks, slot s at
                            #      col (s//3)*512 + (s%3)*132
                            psm = ppm.tile([P, 3 * 512], f32, tag="m")
                            scol = lambda s: (s // 3) * 512 + (s % 3) * TW
                            # start writes: xr expansion per slot
                            nc.tensor.matmul(out=psm[:, 0:TW], lhsT=ident[:, :],
                                             rhs=xrw, start=True, stop=False)
                            for j in range(g):
                                s = j + 1
                                nc.tensor.matmul(
                                    out=psm[:, scol(s):scol(s) + TW],
                                    lhsT=ohT[:, j * P:(j + 1) * P],
                                    rhs=xrw, start=True, stop=False)
                            # accumulate: + gathered/self table rows
                            nc.tensor.matmul(out=psm[:, 0:TW], lhsT=ident[:, :],
                                             rhs=tabw, start=False, stop=True)
                            for j in range(g):
                                s = j + 1
                                nc.tensor.matmul(
                                    out=psm[:, scol(s):scol(s) + TW],
                                    lhsT=ident[:, :],
                                    rhs=gt[:, j * TW:(j + 1) * TW],
                                    start=False, stop=True)
                            # ---- e per edge: two-piece signed abs reduce
                            nb = c // 3          # full banks
                            rem = c - 3 * nb
                            rp = ssp.tile([P, CWMAX], f32, tag="rp")
                            rn = ssp.tile([P, CWMAX], f32, tag="rn")
                            e0 = ssp.tile([P, CWMAX], f32, tag="e0")
                            e1 = ssp.tile([P, CWMAX], f32, tag="e1")
                            base = psm[:, :]

                            def red(dst_ap, a, b):
                                # reduce |psm[:, :, a:b]| over full banks
                                # and the remainder bank
                                if nb > 0:
                                    nc.vector.tensor_reduce(
                                        out=bass.AP(dst_ap.tensor,
                                                    dst_ap.offset,
                                                    [list(dst_ap.ap[0]),
                                                     [3, nb], [1, 3]]),
                                        in_=psm_view(base, nb, 3, a, b - a),
                                        axis=mybir.AxisListType.X, op=OP.add,
                                        apply_absolute_value=True)
                                if rem > 0:
                                    off = nb * 512
                                    nc.vector.tensor_reduce(
                                        out=bass.AP(dst_ap.tensor,
                                                    dst_ap.offset + 3 * nb,
                                                    [list(dst_ap.ap[0]),
                                                     [1, rem]]),
                                        in_=bass.AP(base.tensor,
                                                    base.offset + off + a,
                                                    [list(base.ap[0]),
                                                     [TW, rem], [1, b - a]]),
                                        axis=mybir.AxisListType.X, op=OP.add,
                                        apply_absolute_value=True)

                            red(rp[:, :], 0, Pp)
                            if Pp < H:
                                red(rn[:, :], Pp, H)
                                nc.vector.tensor_tensor(
                                    out=e0[:, :c], in0=rp[:, :c],
                                    in1=rn[:, :c], op=OP.subtract)
                            else:
                                nc.vector.tensor_copy(e0[:, :c], rp[:, :c])
                            # e1 = 1.5*pl + e0  (pl = psm col H per slot)
                            if nb > 0:
                                nc.vector.scalar_tensor_tensor(
                                    out=e1[:, :3 * nb],
                                    in0=psm_view(base, nb, 3, H, 1),
                                    scalar=1.5, in1=e0[:, :3 * nb],
                                    op0=OP.mult, op1=OP.add)
                            if rem > 0:
                                nc.vector.scalar_tensor_tensor(
                                    out=e1[:, 3 * nb:c],
                                    in0=bass.AP(base.tensor,
                                                base.offset + nb * 512 + H,
                                                [list(base.ap[0]),
                                                 [TW, rem], [1, 1]]),
                                    scalar=1.5, in1=e0[:, 3 * nb:c],
                                    op0=OP.mult, op1=OP.add)
                            av = ssp.tile([P, CWMAX], f16, tag="av")
                            nc.scalar.activation(out=av[:, :c], in_=e1[:, :c],
                                                 func=AF.Exp, scale=0.4)
                            # ---- alpha-scaled one-hot + aggregation
                            oha = ohp.tile([P, CWMAX * P], f16, tag="oha")
                            nc.vector.tensor_tensor(
                                out=oha[:, :c * P].rearrange(
                                    "p (c j) -> p c j", j=P),
                                in0=oh3, in1=bcast_last(av[:, :c], c, P),
                                op=OP.mult)
                            pso = ppo.tile([P, H + 2], f32, tag="out")
                            nc.tensor.matmul(out=pso[:, :],
                                             lhsT=oha[:, 0:P],
                                             rhs=tabw[:, 0:H + 2],
                                             start=True, stop=(c == 1))
                            for j in range(g):
                                nc.tensor.matmul(
                                    out=pso[:, :],
                                    lhsT=oha[:, (j + 1) * P:(j + 2) * P],
                                    rhs=gt[:, j * TW:j * TW + H + 2],
                                    start=False, stop=(j == g - 1))
                            # ---- normalize + bias + leaky relu (on DVE)
                            dcl = ssp.tile([P, 1], f32, tag="dcl")
                            nc.vector.tensor_scalar_max(dcl[:, :],
                                                        pso[:, H + 1:H + 2],
                                                        1e-20)
                            rd = ssp.tile([P, 1], f32, tag="rd")
                            nc.vector.reciprocal(rd[:, :], dcl[:, :])
                            h1 = ssp.tile([P, H], f32, tag="h1")
                            nc.vector.scalar_tensor_tensor(
                                out=h1[:, :], in0=pso[:, 0:H], scalar=rd[:, :],
                                in1=arecB[:, :], op0=OP.mult, op1=OP.mult)
                            h2 = ssp.tile([P, H], f32, tag="h2")
                            nc.vector.tensor_tensor(
                                out=h2[:, :], in0=h1[:, :], in1=biasB[:, :],
                                op=OP.add)
                            hw_ = ssp.tile([P, H], f16, tag="hw")
                            nc.vector.scalar_tensor_tensor(
                                out=hw_[:, :], in0=h2[:, :], scalar=0.01,
                                in1=h2[:, :], op0=OP.mult, op1=OP.max)
                            h_out_cb(w, hw_, ssp, extra_psum)

                # ---- layer 1: transpose h into h2T_sb
                with tc.tile_pool(name="pt1", bufs=2, space="PSUM") as ppt:
                    def l1_out(w, hw_, ssp, _):
                        psT = ppt.tile([P, P], f16, tag="tr")
                        nc.tensor.transpose(out=psT[:, :], in_=hw_[:, :],
                                            identity=ident[:, :])
                        nc.vector.tensor_copy(h2T_sb[:, w * P:(w + 1) * P],
                                              psT[:, :])

                    edge_phase(1, d_tab1full, meta["Pp1"], l1_out, None)

                if DEBUG:
                    nc.sync.dma_start(out=d_dbg_h2T[:, :], in_=h2T_sb[:, :])

                # ---- layer 2 phase A from h2T (no loads, no nfc)
                phase_A(2, lambda w: h2T_sb[:, w * P:(w + 1) * P], d_tab2full)

                # ---- layer 2 edge phase + pooling accumulate
                with tc.tile_pool(name="gps", bufs=1, space="PSUM") as gpsp:
                    ps_gs = gpsp.tile([P, G], f32, tag="gs")

                    def l2_out(w, hw_, ssp, _):
                        nc.tensor.matmul(out=ps_gs[:, :], lhsT=hw_[:, :],
                                         rhs=gmask_sb[:, w * G:(w + 1) * G],
                                         start=(w == 0), stop=(w == NWC - 1))

                    edge_phase(2, d_tab2full, meta["Pp2"], l2_out, None)

                    # ---- global mean pool + FC head (replicated)
                    with (
                        tc.tile_pool(name="fc", bufs=1) as fp,
                        tc.tile_pool(name="fcps", bufs=1, space="PSUM") as fpp,
                    ):
                        gsum = fp.tile([P, G], f32)
                        nc.vector.tensor_copy(gsum[:, :], ps_gs[:, :])
                        nc.sync.dma_start(out=d_gsin[:, :], in_=gsum[:, :])
                        nc.gpsimd.collective_compute(
                            "AllReduce", OP.add,
                            replica_groups=[list(range(NCORES))],
                            ins=[d_gsin[:, :]], outs=[d_gsout[:, :]])
                        gsum2 = fp.tile([P, G], f32)
                        nc.sync.dma_start(out=gsum2[:, :], in_=d_gsout[:, :])
                        meanT = fp.tile([P, G], f16)
                        nc.vector.tensor_tensor(out=meanT[:, :], in0=gsum2[:, :],
                                                in1=crecip_sb[:, :], op=OP.mult)
                        psf = fpp.tile([P, 32], f32, tag="f1")
                        nc.tensor.matmul(out=psf[:G, :], lhsT=meanT[:, :G],
                                         rhs=w_sb["wfc1"][:, :],
                                         start=True, stop=False)
                        nc.tensor.matmul(out=psf[:G, :], lhsT=ones1[:, :G],
                                         rhs=w_sb["bfc1"][:, :],
                                         start=False, stop=True)
                        hf1 = fp.tile([P, 32], f16)
                        nc.scalar.activation(out=hf1[:G, :], in_=psf[:G, :],
                                             func=AF.Lrelu, alpha=0.01)
                        psT = fpp.tile([P, P], f16, tag="ft")
                        nc.tensor.transpose(out=psT[:32, :G], in_=hf1[:G, :32],
                                            identity=ident[:G, :G])
                        hf1T = fp.tile([32, P], f16)
                        nc.scalar.activation(out=hf1T[:, :G], in_=psT[:32, :G],
                                             func=AF.Copy)
                        pso = fpp.tile([P, DOUT], f32, tag="f2")
                        nc.tensor.matmul(out=pso[:G, :], lhsT=hf1T[:, :G],
                                         rhs=w_sb["wfc2"][:, :],
                                         start=True, stop=False)
                        nc.tensor.matmul(out=pso[:G, :], lhsT=ones1[:, :G],
                                         rhs=w_sb["bfc2"][:, :],
                                         start=False, stop=True)
                        fout = fp.tile([P, DOUT], f32)
                        nc.vector.tensor_copy(fout[:G, :], pso[:G, :])
                        nc.sync.dma_start(out=d_out[:, :], in_=fout[:G, :])

    nc.compile()
    return nc


# ---------------------------------------------------------------------------
# runner
# ---------------------------------------------------------------------------

def _in_maps(meta, data):
    cfg = meta["cfg"]
    maps = []
    for c in range(cfg["NCORES"]):
        m = {
            "xTloc": data["xTloc"][c],
            "src_idx": data["src_idx"][c],
            "dst_rel": data["dst_rel"][c],
            "gmask": data["gmask"][c],
            "countsRecipB": data["countsRecipB"],
        }
        for k, v in data["weights"].items():
            m[k] = np.ascontiguousarray(v)
        maps.append(m)
    return maps


def run_on_device(inputs, cfg, trace=False):
    from concourse.bass_utils import run_bass_kernel_spmd
    meta, data = host_prep(inputs, cfg)
    nc = build_program(meta)
    res = run_bass_kernel_spmd(nc, _in_maps(meta, data),
                               core_ids=list(range(cfg["NCORES"])), trace=trace)
    return res


def kernel(**inputs):
    res = run_on_device(inputs, FULL_CFG, trace=False)
    return np.asarray(res.results[0]["out"], np.float32)
